# revision 8
# baseline (speedup 1.0000x reference)
"""Trainium2 Bass kernel for a transformer decoder block (self-attn + cross-attn + MLP, post-LN).

Sharding: 8 cores = 2 batches x 4 query-slices of 512 tokens. No collectives:
each core receives the full-batch activations (transposed) to compute K/V
redundantly within its batch group, plus its own query slice. Host stitches
the 8 (512, 1024) outputs back into (2, 2048, 1024).
"""

import os
import sys

import numpy as np

for _p in ("/opt/trn_rl_repo", "/root/.axon_site/_ro/trn_rl_repo"):
    if os.path.isdir(_p) and _p not in sys.path:
        sys.path.append(_p)

import concourse.bass as bass
import concourse.mybir as mybir
import concourse.tile as tile
from concourse import bacc
from concourse.bass_utils import run_bass_kernel_spmd
from concourse.masks import make_identity

B, S, D = 2, 2048, 1024
H, DH = 16, 64
INNER = H * DH
FF = 4096
SCALE = DH ** -0.5
EPS = 1e-5
QS = 512            # queries per core
P = 128
NEG = -1e9          # additive mask value
NCORES = 8

F32 = mybir.dt.float32
F32R = mybir.dt.float32r
AF = mybir.ActivationFunctionType
ALU = mybir.AluOpType


def r(ap):
    return ap


def _dram_bcast(vec_ap, parts):
    """AP that reads a 1-D DRAM vector replicated across `parts` partitions."""
    return bass.AP(
        tensor=vec_ap.tensor,
        offset=vec_ap.offset,
        ap=[[0, parts]] + list(vec_ap.ap),
    )


def _proj_qT(nc, tc, qt_sb, w_dram, rhs_fn, wtag):
    """QT[m-chunk] = sum_d w[d, m].T @ rhs[d]  ->  qt_sb [128, 8, 512]."""
    with tc.tile_pool(name=f"qp_{wtag}", bufs=1) as pool, \
         tc.tile_pool(name=f"qpps_{wtag}", bufs=1, space="PSUM") as psp:
        w_re = w_dram.rearrange("(c p) n -> p c n", p=P)
        for m in range(8):
            w_m = pool.tile([P, 8, P], F32R, tag="w_m", bufs=3, name=f"w_{wtag}_{m}")
            nc.sync.dma_start(out=w_m, in_=w_re[:, :, m * P:(m + 1) * P])
            ps = psp.tile([P, QS], F32, tag="ps", bufs=2, name=f"psq_{wtag}_{m}")
            for d in range(8):
                nc.tensor.matmul(ps, r(w_m[:, d, :]), r(rhs_fn(d)),
                                 start=(d == 0), stop=(d == 7))
            nc.vector.tensor_copy(out=qt_sb[:, m, :], in_=ps)


def _attention_block(nc, tc, *, name, srcT_dram, qt_sb, ot_sb, wk_dram,
                     wv_dram, mask_sb, ones_sb, ones_f32):
    """K/V projection from srcT (DRAM, [1024, 2048]) + S^T-layout attention with
    softmax denominator via a ones-column of V. Writes normalized O^T into
    ot_sb [128, 8, 512]."""
    wk_re = wk_dram.rearrange("(c p) n -> p c n", p=P)
    wv_re = wv_dram.rearrange("(c p) n -> p c n", p=P)
    srcT_re = srcT_dram.rearrange("(c p) s -> p c s", p=P)

    NSL = 256                      # srcT seq-slice width for K/V projection
    NSLICES = S // NSL             # 8

    for hf in range(2):            # head halves (8 heads each)
        hi = hf * 512
        with tc.tile_pool(name=f"{name}_at{hf}", bufs=1) as atp:
            kt_sb = atp.tile([P, 4, S], F32R, tag="kt", name=f"{name}_kt{hf}")
            v_sb = atp.tile([P, 16, 8, 65], F32R, tag="v", name=f"{name}_v{hf}")

            # ---- K^T and V projections over the full sequence ----
            with tc.tile_pool(name=f"{name}_kv{hf}", bufs=1) as kvp, \
                 tc.tile_pool(name=f"{name}_kvps{hf}", bufs=1,
                              space="PSUM") as kvps:
                wk_h = kvp.tile([P, 8, 512], F32R, tag="wk", name=f"{name}_wk{hf}")
                wv_h = kvp.tile([P, 8, 512], F32R, tag="wv", name=f"{name}_wv{hf}")
                nc.sync.dma_start(out=wk_h, in_=wk_re[:, :, hi:hi + 512])
                nc.sync.dma_start(out=wv_h, in_=wv_re[:, :, hi:hi + 512])
                for s in range(NSLICES):
                    xf_s = kvp.tile([P, 8, NSL], F32R, tag="xf", bufs=2,
                                    name=f"{name}_xf{hf}_{s}")
                    nc.sync.dma_start(out=xf_s,
                                      in_=srcT_re[:, :, s * NSL:(s + 1) * NSL])
                    for m in range(4):
                        ps = kvps.tile([P, NSL], F32, tag="psk", bufs=2,
                                       name=f"{name}_psk{hf}_{s}_{m}")
                        for d in range(8):
                            nc.tensor.matmul(ps, r(wk_h[:, d, m * P:(m + 1) * P]),
                                             r(xf_s[:, d, :]),
                                             start=(d == 0), stop=(d == 7))
                        nc.vector.tensor_copy(
                            out=kt_sb[:, m, s * NSL:(s + 1) * NSL], in_=ps)
                    for c in range(NSL // P):
                        ps = kvps.tile([P, 512], F32, tag="psv", bufs=2,
                                       name=f"{name}_psv{hf}_{s}_{c}")
                        for d in range(8):
                            nc.tensor.matmul(ps, r(xf_s[:, d, c * P:(c + 1) * P]),
                                             r(wv_h[:, d, :]),
                                             start=(d == 0), stop=(d == 7))
                        nc.vector.tensor_copy(
                            out=v_sb[:, s * (NSL // P) + c, :, 0:64],
                            in_=ps.rearrange("p (h e) -> p h e", h=8))
            # ones column for the softmax denominator (DVE copy rounds to f32r)
            nc.vector.tensor_copy(
                out=v_sb[:, :, :, 64:65],
                in_=ones_f32[:, 0:128].rearrange("p (a b c) -> p a b c", a=16, b=8))

            # ---- attention for the 8 heads of this half ----
            with tc.tile_pool(name=f"{name}_atps{hf}", bufs=1,
                              space="PSUM") as atps:
                psum_o = {}
                prev = None          # pending AV step (software-pipeline lag 1)
                pending_div = None

                def divide_chain(h_loc, hf=hf, atps=atps):
                    h_glob = hf * 8 + h_loc
                    g = h_glob // 2
                    poff = (h_glob % 2) * 64
                    po = psum_o.pop(h_loc)
                    rcp = atp.tile([1, QS], F32R, tag="rcp", bufs=2,
                                   name=f"{name}_rcp{hf}_{h_loc}")
                    with nc.allow_low_precision(reason="f32r softmax denom"):
                        nc.vector.reciprocal(rcp, po[64:65, :])
                    psb = atps.tile([64, QS], F32, tag="psb", bufs=2,
                                    name=f"{name}_psb{hf}_{h_loc}")
                    nc.tensor.matmul(psb, r(ones_sb[0:1, :]), r(rcp),
                                     start=True, stop=True)
                    rb = atp.tile([64, QS], F32, tag="rb", bufs=2,
                                  name=f"{name}_rb{hf}_{h_loc}")
                    nc.vector.tensor_copy(out=rb, in_=psb)
                    if poff == 0:
                        nc.vector.tensor_tensor(
                            out=ot_sb[0:64, g, :], in0=po[0:64, :], in1=rb,
                            op=ALU.mult)
                    else:
                        tmp = atp.tile([64, QS], F32R, tag="tdiv", bufs=2,
                                       name=f"{name}_td{hf}_{h_loc}")
                        nc.vector.tensor_tensor(out=tmp, in0=po[0:64, :],
                                                in1=rb, op=ALU.mult)
                        nc.sync.dma_start(out=ot_sb[64:128, g, :], in_=tmp)

                for h_loc in range(8):
                    h_glob = hf * 8 + h_loc
                    m_loc = h_loc // 2
                    poff = (h_loc % 2) * 64
                    g = h_glob // 2
                    psum_o[h_loc] = atps.tile([65, QS], F32, tag="pso", bufs=2,
                                              name=f"{name}_pso{hf}_{h_loc}")
                    for j in range(16):
                        ps_s = atps.tile([P, QS], F32, tag="pss", bufs=2,
                                         name=f"{name}_pss{hf}_{h_loc}_{j}")
                        nc.tensor.matmul(
                            ps_s,
                            r(kt_sb[poff:poff + 64, m_loc, j * P:(j + 1) * P]),
                            r(qt_sb[poff:poff + 64, g, :]),
                            start=True, stop=True, tile_position=(poff, 0))
                        et = atp.tile([P, QS], F32R, tag="et", bufs=3,
                                      name=f"{name}_et{hf}_{h_loc}_{j}")
                        nc.scalar.activation(out=et, in_=ps_s, func=AF.Exp,
                                             bias=mask_sb[:, j:j + 1], scale=1.0)
                        if prev is not None:
                            ph, pj, pet = prev
                            nc.tensor.matmul(psum_o[ph], r(v_sb[:, pj, ph, :]),
                                             r(pet), start=(pj == 0),
                                             stop=(pj == 15))
                        prev = (h_loc, j, et)
                        if pending_div is not None and j == 4:
                            divide_chain(pending_div)
                            pending_div = None
                    if h_loc == 7:
                        ph, pj, pet = prev
                        nc.tensor.matmul(psum_o[ph], r(v_sb[:, pj, ph, :]),
                                         r(pet), start=(pj == 0), stop=(pj == 15))
                        prev = None
                        if pending_div is not None:
                            divide_chain(pending_div)
                        divide_chain(7)
                        pending_div = None
                    else:
                        pending_div = h_loc


def _layernorm(nc, pool, name, h_sb, g_sb, b_sb, dst_fn, eps_sb):
    """Row LN over D=1024 per q-chunk: dst_fn(q) = (h-mean)*rstd*g + b."""
    for q in range(4):
        stats = pool.tile([P, 2, 6], F32, tag="lnst", bufs=2, name=f"{name}_st{q}")
        for sub in range(2):
            nc.vector.bn_stats(out=stats[:, sub, :],
                               in_=h_sb[:, q, sub * 512:(sub + 1) * 512])
        mv = pool.tile([P, 2], F32, tag="lnmv", bufs=2, name=f"{name}_mv{q}")
        nc.vector.bn_aggr(out=mv, in_=stats)
        std = pool.tile([P, 1], F32, tag="lnstd", bufs=2, name=f"{name}_sd{q}")
        nc.scalar.activation(out=std, in_=mv[:, 1:2], func=AF.Sqrt, bias=eps_sb)
        rstd = pool.tile([P, 1], F32, tag="lnrs", bufs=2, name=f"{name}_rs{q}")
        nc.vector.reciprocal(rstd, std)
        dst = dst_fn(q)
        nc.vector.tensor_scalar(out=dst, in0=h_sb[:, q, :],
                                scalar1=mv[:, 0:1], scalar2=rstd,
                                op0=ALU.subtract, op1=ALU.mult)
        nc.vector.tensor_tensor(out=dst, in0=dst, in1=g_sb, op=ALU.mult)
        nc.vector.tensor_tensor(out=dst, in0=dst, in1=b_sb, op=ALU.add)


def _out_proj_residual_ln(nc, tc, t, *, name, ot_sb, wo_dram, res_fn,
                          extra_bias, g_name, b_name, dst_sb, eps_sb):
    """dst_sb[:, q, :] = LN(res(q) + (O^T)^T @ wo [+ extra_bias]) for 4 q-chunks."""
    wo_re = wo_dram.rearrange("(c p) n -> p c n", p=P)
    with tc.tile_pool(name=f"{name}_wo", bufs=1) as wop, \
         tc.tile_pool(name=f"{name}_wops", bufs=1, space="PSUM") as psp:
        g_sb = wop.tile([P, 1024], F32, tag="g", name=f"{name}_g")
        nc.gpsimd.dma_start(out=g_sb, in_=_dram_bcast(t[g_name], P))
        b_sb = wop.tile([P, 1024], F32, tag="b", name=f"{name}_b")
        nc.gpsimd.dma_start(out=b_sb, in_=_dram_bcast(t[b_name], P))
        eb_sb = None
        if extra_bias is not None:
            eb_sb = wop.tile([P, 1024], F32, tag="eb", name=f"{name}_eb")
            nc.gpsimd.dma_start(out=eb_sb, in_=_dram_bcast(t[extra_bias], P))

        ps = [[psp.tile([P, 512], F32, tag=f"pp{q}_{n}", name=f"{name}_pp{q}_{n}")
               for n in range(2)] for q in range(4)]
        for c in range(8):
            wo_c = wop.tile([P, 1024], F32R, tag="wo", bufs=3, name=f"{name}_wo{c}")
            nc.sync.dma_start(out=wo_c, in_=wo_re[:, c, :])
            for n in range(2):
                for q in range(4):
                    nc.tensor.matmul(
                        ps[q][n], r(ot_sb[:, c, q * P:(q + 1) * P]),
                        r(wo_c[:, n * 512:(n + 1) * 512]),
                        start=(c == 0), stop=(c == 7))
        h_sb = wop.tile([P, 4, 1024], F32, tag="h", name=f"{name}_h")
        for q in range(4):
            for n in range(2):
                nc.vector.tensor_tensor(out=h_sb[:, q, n * 512:(n + 1) * 512],
                                        in0=ps[q][n], in1=res_fn(q, n),
                                        op=ALU.add)
            if eb_sb is not None:
                nc.vector.tensor_tensor(out=h_sb[:, q, :], in0=h_sb[:, q, :],
                                        in1=eb_sb, op=ALU.add)
        _layernorm(nc, wop, name, h_sb, g_sb, b_sb,
                   lambda q: dst_sb[:, q, :], eps_sb)


def _transpose_qD(nc, tc, name, src_sb, dst_sb, ident):
    """src [128, 4, 1024] (q-major) -> dst [128, 8, 512] (D-major) via PE."""
    with tc.tile_pool(name=f"{name}_tps", bufs=1, space="PSUM") as psp:
        for c in range(8):
            for q in range(4):
                pt = psp.tile([P, P], F32, tag="pt", bufs=2,
                              name=f"{name}_pt{c}_{q}")
                nc.tensor.matmul(pt, src_sb[:, q, c * P:(c + 1) * P], ident,
                                 is_transpose=True, start=True, stop=True)
                nc.vector.tensor_copy(out=dst_sb[:, c, q * P:(q + 1) * P], in_=pt)


def build_nc():
    nc = bacc.Bacc("TRN2", target_bir_lowering=False, debug=False,
                   enable_asserts=False, num_devices=NCORES)
    t = {}
    def din(name, shape):
        t[name] = nc.dram_tensor(name, list(shape), F32, kind="ExternalInput").ap()
    def dinr(name, shape):
        t[name] = nc.dram_tensor(name, list(shape), F32R,
                                 kind="ExternalInput").ap()
    dinr("xqT", (D, QS)); din("xq", (QS, D))
    dinr("xfT", (D, S)); dinr("efT", (D, S))
    din("tmask", (P, 16)); din("smask", (P, 16))
    for w in ("sa_wq", "sa_wk", "sa_wv", "sa_wo", "ca_wq", "ca_wk", "ca_wv",
              "ca_wo"):
        dinr(w, (D, INNER))
    dinr("fc1_w", (D, FF)); dinr("fc2_w", (FF, D))
    din("fc1_b", (P, 32)); din("fc2_b", (D,)); din("ca_bo", (D,))
    for v in ("ln1_g", "ln1_b", "ln2_g", "ln2_b", "ln3_g", "ln3_b"):
        din(v, (D,))
    out = nc.dram_tensor("out", [QS, D], F32, kind="ExternalOutput").ap()

    with tile.TileContext(nc) as tc:
        with tc.tile_pool(name="perm", bufs=1) as perm, \
             tc.tile_pool(name="keep", bufs=1) as keep:
            ident = perm.tile([P, P], F32, tag="ident", name="ident")
            make_identity(nc, ident)
            ones_f32 = perm.tile([P, P], F32, tag="ones32", name="ones_f32")
            nc.vector.memset(ones_f32, 1.0)
            ones_sb = perm.tile([1, 64], F32R, tag="ones", name="ones")
            nc.vector.tensor_copy(out=ones_sb, in_=ones_f32[0:1, 0:64])
            eps_sb = perm.tile([P, 1], F32, tag="eps", name="eps_sb")
            nc.vector.memset(eps_sb, EPS)
            tmask_sb = perm.tile([P, 16], F32, tag="tmask", name="tmask_sb")
            nc.sync.dma_start(out=tmask_sb, in_=t["tmask"])
            smask_sb = perm.tile([P, 16], F32, tag="smask", name="smask_sb")
            nc.sync.dma_start(out=smask_sb, in_=t["smask"])

            ln1_out = keep.tile([P, 4, 1024], F32, tag="ln1o", name="ln1_out")

            # ================= self-attention =================
            with tc.tile_pool(name="sa_otp", bufs=1) as sao:
                ot_sb = sao.tile([P, 8, QS], F32R, tag="ot", name="sa_ot")
                with tc.tile_pool(name="sa_qtp", bufs=1) as saq:
                    qt_sb = saq.tile([P, 8, QS], F32R, tag="qt", name="sa_qt")
                    with tc.tile_pool(name="sa_qin", bufs=1) as qin:
                        xqT_sb = qin.tile([P, 8, QS], F32R, tag="xqT",
                                          name="xqT_sb")
                        nc.sync.dma_start(
                            out=xqT_sb,
                            in_=t["xqT"].rearrange("(c p) s -> p c s", p=P))
                        _proj_qT(nc, tc, qt_sb, t["sa_wq"],
                                 lambda d: xqT_sb[:, d, :], "saq")
                    _attention_block(nc, tc, name="sa", srcT_dram=t["xfT"],
                                     qt_sb=qt_sb, ot_sb=ot_sb,
                                     wk_dram=t["sa_wk"], wv_dram=t["sa_wv"],
                                     mask_sb=tmask_sb, ones_sb=ones_sb,
                                     ones_f32=ones_f32)
                with tc.tile_pool(name="sa_res", bufs=1) as sar:
                    xq_sb = sar.tile([P, 4, 1024], F32, tag="xq", name="xq_sb")
                    nc.sync.dma_start(
                        out=xq_sb, in_=t["xq"].rearrange("(c p) d -> p c d", p=P))
                    _out_proj_residual_ln(
                        nc, tc, t, name="sawo", ot_sb=ot_sb, wo_dram=t["sa_wo"],
                        res_fn=lambda q, n: xq_sb[:, q, n * 512:(n + 1) * 512],
                        extra_bias=None, g_name="ln1_g", b_name="ln1_b",
                        dst_sb=ln1_out, eps_sb=eps_sb)

            # ================= cross-attention =================
            with tc.tile_pool(name="ca_otp", bufs=1) as cao:
                ot_sb = cao.tile([P, 8, QS], F32R, tag="ot", name="ca_ot")
                with tc.tile_pool(name="ca_qtp", bufs=1) as caq:
                    qt_sb = caq.tile([P, 8, QS], F32R, tag="qt", name="ca_qt")
                    with tc.tile_pool(name="ca_t1", bufs=1) as t1p:
                        ln1T = t1p.tile([P, 8, QS], F32R, tag="ln1T", name="ln1T")
                        _transpose_qD(nc, tc, "t1", ln1_out, ln1T, ident)
                        _proj_qT(nc, tc, qt_sb, t["ca_wq"],
                                 lambda d: ln1T[:, d, :], "caq")
                    _attention_block(nc, tc, name="ca", srcT_dram=t["efT"],
                                     qt_sb=qt_sb, ot_sb=ot_sb,
                                     wk_dram=t["ca_wk"], wv_dram=t["ca_wv"],
                                     mask_sb=smask_sb, ones_sb=ones_sb,
                                     ones_f32=ones_f32)
                # ---- ln2_out lives from CA out-proj through the MLP ----
                with tc.tile_pool(name="mid", bufs=1) as mid:
                    ln2_out = mid.tile([P, 4, 1024], F32, tag="ln2o",
                                       name="ln2_out")
                    _out_proj_residual_ln(
                        nc, tc, t, name="cawo", ot_sb=ot_sb, wo_dram=t["ca_wo"],
                        res_fn=lambda q, n: ln1_out[:, q, n * 512:(n + 1) * 512],
                        extra_bias="ca_bo", g_name="ln2_g", b_name="ln2_b",
                        dst_sb=ln2_out, eps_sb=eps_sb)

                    # ================= MLP =================
                    with tc.tile_pool(name="mlp", bufs=1) as mlp:
                        ln2T = mlp.tile([P, 8, QS], F32R, tag="ln2T", name="ln2T")
                        _transpose_qD(nc, tc, "t2", ln2_out, ln2T, ident)
                        h2T = mlp.tile([P, 32, QS], F32R, tag="h2T", name="h2T")
                        fc1b_sb = mlp.tile([P, 32], F32, tag="f1b",
                                           name="fc1b_sb")
                        nc.sync.dma_start(out=fc1b_sb, in_=t["fc1_b"])
                        fc1_re = t["fc1_w"].rearrange("(c p) n -> p c n", p=P)
                        with tc.tile_pool(name="fc1", bufs=1) as f1p, \
                             tc.tile_pool(name="fc1ps", bufs=1,
                                          space="PSUM") as f1ps:
                            for m in range(32):
                                w_m = f1p.tile([P, 8, P], F32R, tag="w", bufs=3,
                                               name=f"fc1w_{m}")
                                nc.sync.dma_start(
                                    out=w_m, in_=fc1_re[:, :, m * P:(m + 1) * P])
                                psf = f1ps.tile([P, QS], F32, tag="ps", bufs=2,
                                                name=f"fc1ps_{m}")
                                for d in range(8):
                                    nc.tensor.matmul(psf, r(w_m[:, d, :]),
                                                     r(ln2T[:, d, :]),
                                                     start=(d == 0), stop=(d == 7))
                                nc.scalar.activation(
                                    out=h2T[:, m, :], in_=psf, func=AF.Gelu,
                                    bias=fc1b_sb[:, m:m + 1], scale=1.0)
                        fc2_re = t["fc2_w"].rearrange("(c p) n -> p c n", p=P)
                        with tc.tile_pool(name="fc2", bufs=1) as f2p, \
                             tc.tile_pool(name="fc2ps", bufs=1,
                                          space="PSUM") as f2ps:
                            fc2b_sb = f2p.tile([P, 1024], F32, tag="f2b",
                                               name="fc2b_sb")
                            nc.gpsimd.dma_start(out=fc2b_sb,
                                                in_=_dram_bcast(t["fc2_b"], P))
                            ln3g_sb = f2p.tile([P, 1024], F32, tag="l3g",
                                               name="ln3g_sb")
                            nc.gpsimd.dma_start(out=ln3g_sb,
                                                in_=_dram_bcast(t["ln3_g"], P))
                            ln3b_sb = f2p.tile([P, 1024], F32, tag="l3b",
                                               name="ln3b_sb")
                            nc.gpsimd.dma_start(out=ln3b_sb,
                                                in_=_dram_bcast(t["ln3_b"], P))
                            ps = [[f2ps.tile([P, 512], F32, tag=f"pr{q}_{n}",
                                             name=f"fc2ps{q}_{n}")
                                   for n in range(2)] for q in range(4)]
                            for fb in range(8):
                                wb = f2p.tile([P, 4, 1024], F32R, tag="wb",
                                              bufs=2, name=f"fc2w_{fb}")
                                nc.sync.dma_start(
                                    out=wb,
                                    in_=fc2_re[:, fb * 4:(fb + 1) * 4, :])
                                for q in range(4):
                                    for n in range(2):
                                        for f in range(4):
                                            ff = fb * 4 + f
                                            nc.tensor.matmul(
                                                ps[q][n],
                                                r(h2T[:, ff, q * P:(q + 1) * P]),
                                                r(wb[:, f, n * 512:(n + 1) * 512]),
                                                start=(ff == 0), stop=(ff == 31))
                            # residual in place into ln2_out, then LN3 -> out
                            for q in range(4):
                                for n in range(2):
                                    nc.vector.tensor_tensor(
                                        out=ln2_out[:, q, n * 512:(n + 1) * 512],
                                        in0=ps[q][n],
                                        in1=ln2_out[:, q, n * 512:(n + 1) * 512],
                                        op=ALU.add)
                                nc.vector.tensor_tensor(
                                    out=ln2_out[:, q, :], in0=ln2_out[:, q, :],
                                    in1=fc2b_sb, op=ALU.add)
                            out_re = out.rearrange("(c p) d -> p c d", p=P)
                            # LN3 with per-q output tiles DMA'd straight out
                            for q in range(4):
                                o_sb = f2p.tile([P, 1024], F32, tag="osb",
                                                bufs=2, name=f"out_sb{q}")
                                _layernorm_q(nc, f2p, f"ln3_{q}", ln2_out, q,
                                             ln3g_sb, ln3b_sb, o_sb, eps_sb)
                                nc.sync.dma_start(out=out_re[:, q, :], in_=o_sb)
    nc.compile()
    return nc


def _layernorm_q(nc, pool, name, h_sb, q, g_sb, b_sb, dst, eps_sb):
    stats = pool.tile([P, 2, 6], F32, tag="lnst", bufs=2, name=f"{name}_st")
    for sub in range(2):
        nc.vector.bn_stats(out=stats[:, sub, :],
                           in_=h_sb[:, q, sub * 512:(sub + 1) * 512])
    mv = pool.tile([P, 2], F32, tag="lnmv", bufs=2, name=f"{name}_mv")
    nc.vector.bn_aggr(out=mv, in_=stats)
    std = pool.tile([P, 1], F32, tag="lnstd", bufs=2, name=f"{name}_sd")
    nc.scalar.activation(out=std, in_=mv[:, 1:2], func=AF.Sqrt, bias=eps_sb)
    rstd = pool.tile([P, 1], F32, tag="lnrs", bufs=2, name=f"{name}_rs")
    nc.vector.reciprocal(rstd, std)
    nc.vector.tensor_scalar(out=dst, in0=h_sb[:, q, :],
                            scalar1=mv[:, 0:1], scalar2=rstd,
                            op0=ALU.subtract, op1=ALU.mult)
    nc.vector.tensor_tensor(out=dst, in0=dst, in1=g_sb, op=ALU.mult)
    nc.vector.tensor_tensor(out=dst, in0=dst, in1=b_sb, op=ALU.add)


_NC_CACHE = None


def _get_nc():
    global _NC_CACHE
    if _NC_CACHE is None:
        _NC_CACHE = build_nc()
    return _NC_CACHE


def make_in_maps(inputs):
    f32c = lambda a: np.ascontiguousarray(np.asarray(a), dtype=np.float32)
    x = f32c(inputs["x"]); enc = f32c(inputs["enc_output"])
    tgt = np.asarray(inputs["tgt_mask"]); src = np.asarray(inputs["src_mask"])

    shared = {
        "sa_wq": f32c(inputs["sa_wq"]) * np.float32(SCALE),
        "sa_wk": f32c(inputs["sa_wk"]), "sa_wv": f32c(inputs["sa_wv"]),
        "sa_wo": f32c(inputs["sa_wo"]),
        "ca_wq": f32c(inputs["ca_wq"]) * np.float32(SCALE),
        "ca_wk": f32c(inputs["ca_wk"]), "ca_wv": f32c(inputs["ca_wv"]),
        "ca_wo": f32c(inputs["ca_wo"]),
        "fc1_w": f32c(inputs["fc1_w"]), "fc2_w": f32c(inputs["fc2_w"]),
        "fc1_b": np.ascontiguousarray(f32c(inputs["fc1_b"]).reshape(32, P).T),
        "fc2_b": f32c(inputs["fc2_b"]), "ca_bo": f32c(inputs["ca_bo"]),
        "ln1_g": f32c(inputs["ln1_g"]), "ln1_b": f32c(inputs["ln1_b"]),
        "ln2_g": f32c(inputs["ln2_g"]), "ln2_b": f32c(inputs["ln2_b"]),
        "ln3_g": f32c(inputs["ln3_g"]), "ln3_b": f32c(inputs["ln3_b"]),
    }
    shared = {k: np.ascontiguousarray(v) for k, v in shared.items()}

    in_maps = []
    for c in range(NCORES):
        b, qi = c // 4, c % 4
        q0 = qi * QS
        xb = x[b]; eb = enc[b]
        tm = np.where(tgt[b], np.float32(NEG), np.float32(0.0)).astype(np.float32)
        sm = np.where(src[b], np.float32(NEG), np.float32(0.0)).astype(np.float32)
        m = dict(shared)
        m["xqT"] = np.ascontiguousarray(xb[q0:q0 + QS].T)
        m["xq"] = np.ascontiguousarray(xb[q0:q0 + QS] + f32c(inputs["sa_bo"]))
        m["xfT"] = np.ascontiguousarray(xb.T)
        m["efT"] = np.ascontiguousarray(eb.T)
        m["tmask"] = np.ascontiguousarray(tm.reshape(16, P).T)
        m["smask"] = np.ascontiguousarray(sm.reshape(16, P).T)
        in_maps.append(m)
    return in_maps


def kernel(**inputs):
    nc = _get_nc()
    in_maps = make_in_maps(inputs)
    res = run_bass_kernel_spmd(nc, in_maps, core_ids=list(range(NCORES)))
    out = np.empty((B, S, D), np.float32)
    for c in range(NCORES):
        b, qi = c // 4, c % 4
        out[b, qi * QS:(qi + 1) * QS] = res.results[c]["out"]
    return out
